# revision 5
# baseline (speedup 1.0000x reference)
"""Trainium2 Bass kernel for the BANLayer problem (v3).

Computation (per batch):
    Uc   = relu(h_c @ U_w.T + U_b)            # (N, D)
    Vp   = relu(h_p @ V_w.T + V_b)            # (M, D)
    attn = Uc @ Vp.T                          # (N, M)
    w    = softmax(attn, axis=-1)
    ctx  = w @ Vp                             # (N, D)
    out  = mean_n((Uc + ctx) * q)             # (B, D)

Device algorithm (data-parallel over batch, 8 cores x 8 batches):
    mean-factorized:  out = (q/N) * (sum_n Uc[n,:] + sum_m c[m] * Vp[m,:])
    with  c[m] = sum_n E[n,m] / s[n],  E = exp(attn - 40),  s[n] = sum_m E.

    Engine balance (per batch, ~2.9us each):
      ACT : exp chunks 0/2 with fused row-sum accumulator + vpT[0:256] relu
      DVE : magic exp chunk 1 + chunk 3 head + relu(Uc)+row-sum via
            scalar_tensor_tensor accum + reciprocals
      Pool: hp/hc input DMAs + magic chunk 3 tail + vpT[256:1024] relu +
            small converts (rbf/cbf/y)
      SP  : the three DMA-xbar transposes (vp2, et1, et3)
      PE  : projections + attention + all reductions as 1-col matmuls

    The magic exp: one fused tensor_scalar u = attn*A + B with int16
    output -- the 16-bit integers ARE the bf16 bit patterns of
    exp(attn-40) (max rel err ~3.5%, self-normalizing through softmax).
    Row sums of magic chunks come from DMA xbar transposes of the bf16
    tiles followed by 1-column PE matmuls against ones; c[m] and
    yctx[e] use the same 1-column-matmul trick so those reductions cost
    no ACT/DVE/Pool time.
"""

import sys

import numpy as np

sys.path.insert(0, "/opt/trn_rl_repo")

B, N, M, D = 64, 512, 1024, 128
CORES = 8
BL = B // CORES
SHIFT = 40.0    # softmax logit shift (exact by shift invariance)
NCH = N // 128  # n-chunks per batch
MH = M // 512   # m-halves per attention matmul
NMAGIC = 2      # chunks 1,3 use the magic exp
NACT = NCH - NMAGIC
XSPL = 688      # chunk-3 magic split: DVE [0:XSPL], Pool [XSPL:1024]
VSPL = 256      # vpT relu split: ACT [0:VSPL], Pool [VSPL:1024]

_LN2 = float(np.log(2.0))
A2 = float(np.float32(128.0 / _LN2))
# bf16 Schraudolph bias: 127*2^7 centered (c_adj=6) + 0.5 trunc->round
# compensation, with the logit shift folded in: t16 = A2*attn + B2.
B2 = float(np.float32(np.float32(127 * 128 - 6 + 0.5)
                      - np.float32(SHIFT) * np.float32(A2)))

_BUILT = {}


def _build_nc():
    import concourse.bass as bass  # noqa: F401
    import concourse.tile as tile
    from concourse import bacc, mybir

    F32 = mybir.dt.float32
    BF16 = mybir.dt.bfloat16
    I16 = mybir.dt.int16
    OP = mybir.AluOpType
    ACTF = mybir.ActivationFunctionType

    nc = bacc.Bacc("TRN2", target_bir_lowering=False, debug=False,
                   num_devices=CORES)

    hcT = nc.declare_dram_parameter("hcT", [BL, D, N], BF16, isOutput=False)
    hpT = nc.declare_dram_parameter("hpT", [BL, D, M], BF16, isOutput=False)
    w2 = nc.declare_dram_parameter("w2", [D, 2 * D], BF16, isOutput=False)
    bias3 = nc.declare_dram_parameter("bias3", [D, 3], F32, isOutput=False)
    y = nc.declare_dram_parameter("y", [D, BL], F32, isOutput=True)

    with tile.TileContext(nc) as tc:
        with (
            tc.tile_pool(name="consts", bufs=1) as consts,
            tc.tile_pool(name="inp", bufs=4) as inp,
            tc.tile_pool(name="proj", bufs=3) as proj,
            tc.tile_pool(name="epool", bufs=3 * NACT) as epool,
            tc.tile_pool(name="upool", bufs=3) as upool,
            tc.tile_pool(name="tpool", bufs=9) as tpool,
            tc.tile_pool(name="stats", bufs=4) as stats,
            tc.tile_pool(name="psA", bufs=2, space="PSUM") as psA,
            tc.tile_pool(name="psU", bufs=1, space="PSUM") as psU,
            tc.tile_pool(name="psV", bufs=1, space="PSUM") as psV,
            tc.tile_pool(name="psS", bufs=1, space="PSUM") as psS,
        ):
            # ---- constants ------------------------------------------------
            w2_sb = consts.tile([D, 2 * D], BF16)
            nc.sync.dma_start(w2_sb[:], w2[:])
            uwT_sb = w2_sb[:, 0:D]
            vwT_sb = w2_sb[:, D:2 * D]
            b3_sb = consts.tile([D, 3], F32)
            nc.gpsimd.dma_start(b3_sb[:], bias3[:])
            ub_sb = b3_sb[:, 0:1]
            vb_sb = b3_sb[:, 1:2]
            qn_sb = b3_sb[:, 2:3]
            nshift = consts.tile([128, 1], F32)
            nc.vector.memset(nshift[:], -SHIFT)
            onesb = consts.tile([128, 1], BF16)
            nc.vector.memset(onesb[:], 1.0)
            zero5 = consts.tile([128, N], BF16)
            nc.vector.memset(zero5[:], 0.0)
            y_sb = consts.tile([D, BL], F32)

            hcs, hps = [], []

            def load_batch(b):
                hc = inp.tile([D, N], BF16, name="hc")
                hp = inp.tile([D, M], BF16, name="hp")
                nc.gpsimd.dma_start(hc[:], hcT[b])
                nc.gpsimd.dma_start(hp[:], hpT[b])
                hcs.append(hc)
                hps.append(hp)

            load_batch(0)
            load_batch(1)

            # ACT exp-table prefetch + PE warmup while the first DMAs land
            warm = stats.tile([128, 1], F32, name="warm")
            nc.scalar.activation(warm[:], nshift[:], ACTF.Exp,
                                 bias=0.0, scale=1.0)
            wu_ps = psS.tile([128, 16], F32, name="small")
            nc.tensor.matmul(wu_ps[0:1, 0:1], onesb[:, 0:1], onesb[:],
                             start=True, stop=True)

            state = {}

            def part_proj(b):
                """Projections + evictions + Vp transpose."""
                hc = hcs[b]
                hp = hps[b]

                uc_ps = psU.tile([128, N], F32, name="uc_ps")
                nc.tensor.matmul(uc_ps[:], uwT_sb, hc[:],
                                 start=True, stop=True)
                ucT = proj.tile([D, N], BF16, name="ucT")
                ucsum = stats.tile([D, 1], F32, name="ucsum")
                # relu(x + b) with fused row-sum accumulator, on DVE
                nc.vector.scalar_tensor_tensor(
                    ucT[:], uc_ps[:], ub_sb, zero5[:],
                    OP.add, OP.max, accum_out=ucsum[:])

                vp_ps = psV.tile([128, M], F32, name="vp_ps")
                vpT = proj.tile([D, M], BF16, name="vpT")
                for h in range(MH):
                    nc.tensor.matmul(vp_ps[:, h * 512:(h + 1) * 512],
                                     vwT_sb,
                                     hp[:, h * 512:(h + 1) * 512],
                                     start=True, stop=True)
                nc.scalar.activation(vpT[:], vp_ps[:],
                                     ACTF.Relu, bias=vb_sb, scale=1.0)

                vp2 = tpool.tile([128, M], BF16, name="vp2")
                nc.sync.dma_start(vp2[:].rearrange("p (c e) -> p c e", c=8),
                                  vpT[:], transpose=True)
                state[b] = {"ucT": ucT, "vpT": vpT, "vp2": vp2,
                            "ucsum": ucsum}

            def part_attn(b):
                """Attention matmuls + exp (ACT native / DVE+Pool magic)."""
                st = state[b]
                ucT, vpT = st["ucT"], st["vpT"]
                s4 = stats.tile([128, NACT], F32, name="s4")
                e_sbs = {}
                u23 = upool.tile([128, NMAGIC * 1024], I16, name="u23")
                ets = []
                # even chunks -> ACT native exp, odd chunks -> magic exp:
                # the consumers drain the att PSUM banks in parallel
                for j in range(NCH):
                    att_ps = psA.tile([128, 1024], F32, name="att_ps")
                    lhs = ucT[:, j * 128:(j + 1) * 128]
                    for h in range(MH):
                        nc.tensor.matmul(att_ps[:, h * 512:(h + 1) * 512],
                                         lhs,
                                         vpT[:, h * 512:(h + 1) * 512],
                                         start=True, stop=True)
                    if j % 2 == 0:
                        e_sb = epool.tile([128, M], BF16, name="e_sb")
                        nc.scalar.activation(e_sb[:], att_ps[:], ACTF.Exp,
                                             bias=nshift[:], scale=1.0,
                                             accum_out=s4[:, j // 2:j // 2 + 1])
                        e_sbs[j] = e_sb
                    else:
                        jj = j // 2
                        base = jj * 1024
                        nc.vector.tensor_scalar(
                            u23[:, base:base + 1024], att_ps[:],
                            A2, B2, OP.mult, OP.add)
                        et = tpool.tile([128, 1024], BF16, name="et")
                        nc.sync.dma_start(
                            et[:].rearrange("p (c e) -> p c e", c=8),
                            u23[:].bitcast(BF16)[:, base:base + 1024],
                            transpose=True)
                        ets.append(et)
                st.update(s4=s4, e_sbs=e_sbs, u23=u23, et=ets)

            def part_s(b):
                """Row sums of magic chunks on PE; r = 1/s on DVE."""
                st = state[b]
                small = psS.tile([128, 16], F32, name="small")
                s_ps = small[:, 0:NMAGIC]
                st["small"] = small

                r4 = stats.tile([128, NCH], F32, name="r4")
                for jj in range(NMAGIC):
                    etv = st["et"][jj][:].rearrange("p (c e) -> p c e", c=8)
                    for mc in range(8):
                        nc.tensor.matmul(s_ps[:, jj:jj + 1],
                                         etv[:, mc, :],
                                         onesb[:],
                                         start=(mc == 0), stop=(mc == 7))
                nc.vector.reciprocal(r4[:, NACT:NCH], s_ps[:])
                nc.vector.reciprocal(r4[:, 0:NACT], st["s4"][:])
                rbf = stats.tile([128, NCH], BF16, name="rbf")
                nc.gpsimd.tensor_scalar(rbf[:], r4[:], 1.0, None, OP.mult)
                st["rbf"] = rbf

            def part_c(b):
                """c[m], then yctx and the final combine."""
                st = state.pop(b)
                small = st["small"]
                c_ps = small[:, NMAGIC:NMAGIC + 8]
                y_ps = small[:, NMAGIC + 8:NMAGIC + 9]
                rbf = st["rbf"]
                u23v = st["u23"][:].bitcast(BF16)

                for mc in range(8):
                    for j in range(NCH):
                        if j % 2 == 0:
                            lhsT = st["e_sbs"][j][:,
                                                  mc * 128:(mc + 1) * 128]
                        else:
                            jj = j // 2
                            lhsT = u23v[:, jj * 1024 + mc * 128:
                                        jj * 1024 + (mc + 1) * 128]
                        rcol = (j // 2) if j % 2 == 0 else (NACT + j // 2)
                        nc.tensor.matmul(c_ps[:, mc:mc + 1], lhsT,
                                         rbf[:, rcol:rcol + 1],
                                         start=(j == 0),
                                         stop=(j == NCH - 1))
                cbf = stats.tile([128, 8], BF16, name="cbf")
                nc.vector.tensor_scalar(cbf[:], c_ps[:], 1.0, None, OP.mult)

                vp2v = st["vp2"][:].rearrange("p (c e) -> p c e", c=8)
                for mc in range(8):
                    nc.tensor.matmul(y_ps[:], vp2v[:, mc, :],
                                     cbf[:, mc:mc + 1],
                                     start=(mc == 0), stop=(mc == 7))

                tsum = stats.tile([D, 1], F32, name="tsum")
                nc.vector.tensor_tensor(tsum[:], st["ucsum"][:], y_ps[:],
                                        OP.add)
                nc.gpsimd.tensor_scalar(y_sb[:, b:b + 1], tsum[:], qn_sb,
                                        None, OP.mult)

            LAG = 2
            part_proj(0)
            part_proj(1)
            for b in range(BL):
                if len(hcs) < min(b + 4, BL):
                    load_batch(len(hcs))
                part_attn(b)
                if b + 2 < BL:
                    part_proj(b + 2)
                if b >= LAG:
                    part_s(b - LAG)
                    part_c(b - LAG)
            for b in range(max(0, BL - LAG), BL):
                part_s(b)
                part_c(b)

            nc.sync.dma_start(y[:], y_sb[:])

    nc.finalize()
    return nc


def kernel(h_c, h_p, U_w, U_b, V_w, V_b, q):
    import ml_dtypes
    from concourse.bass_utils import run_bass_kernel_spmd

    if "nc" not in _BUILT:
        _BUILT["nc"] = _build_nc()
    nc = _BUILT["nc"]

    bf16 = ml_dtypes.bfloat16
    h_c = np.asarray(h_c, dtype=np.float32)
    h_p = np.asarray(h_p, dtype=np.float32)
    w2 = np.ascontiguousarray(np.concatenate(
        [np.asarray(U_w, dtype=np.float32).T,
         np.asarray(V_w, dtype=np.float32).T], axis=1).astype(bf16))
    bias3 = np.ascontiguousarray(np.stack(
        [np.asarray(U_b, dtype=np.float32),
         np.asarray(V_b, dtype=np.float32),
         np.asarray(q, dtype=np.float32) / np.float32(N)], axis=1))

    in_maps = []
    for c in range(CORES):
        sl = slice(c * BL, (c + 1) * BL)
        in_maps.append({
            "hcT": np.ascontiguousarray(
                h_c[sl].transpose(0, 2, 1).astype(bf16)),
            "hpT": np.ascontiguousarray(
                h_p[sl].transpose(0, 2, 1).astype(bf16)),
            "w2": w2, "bias3": bias3,
        })

    global _last_in_maps
    _last_in_maps = in_maps
    res = run_bass_kernel_spmd(nc, in_maps, core_ids=list(range(CORES)))
    out = np.empty((B, D), dtype=np.float32)
    for c in range(CORES):
        out[c * BL:(c + 1) * BL] = res.results[c]["y"].T
    return out


_BUILt = _BUILT  # legacy alias for test.py
